# revision 42
# baseline (speedup 1.0000x reference)
"""CapsuleLayer (dynamic routing) on 8 trn2 NeuronCores.

Math: u_hat[b,c,i,o] = sum_{d,k} W[c,0,i,o,d,k] x[b,i,k]
             = sum_k Wsum[c,i,o,k] x[b,i,k],  Wsum = W.sum(d)   (134MB -> 8.4MB)
Routing logits are cumulative: b_t = u_hat . (sum_{tau<t} v_tau), so each
iteration only needs the running vector-sum w.  Everything is sharded over
IN_CAPS (i) across 8 cores; only s[b,c,o] (131KB) crosses cores via AllReduce.

Per-core layouts (partition dim = i throughout the routing iterations):
  xt_i[h]   [128(i), (k,b)]  bf16  - x transposed via PE + 1MB HBM round-trip
  wf        [128(i), (c,h,(k,o))] bf16 - Wsum, d-reduced on DVE/Pool trees
  T_all     [128(k,o), (c,h,i)]  bf16 - PE-transpose of wf (G stationaries)
  w_acc     [16(o), (c,b)]  bf16 - running sum of squash outputs v
Iteration t:
  G_k[i,b]  = T[c,h,k-slice]^T w_acc[c]          (PE, K=o=16)
  P         = xt_i * G (PSUM f32 read, DVE/Pool), bt = sum_k P (tree adds)
  c_t       = softmax_c(bt)  (exp on Act, den tree, no max-subtraction)
  y_c       = ct_c (bcast over k) * xt_i         (DVE, all bf16)
  s_c[o,b] += wf[c,h,k-slice]^T y_c              (PE, accumulate 16 matmuls)
AllReduce s (131KB f32) -> squash -> w_acc (or v output on last iter).
"""

import contextlib
import sys
import types

import numpy as np
import ml_dtypes  # noqa: F401  (bf16 array dtype for I/O maps)


def _install_ntff_shim():
    """The image's antenv lacks axon_hooks; provide a minimal equivalent so
    run_bass_kernel_spmd(trace=True) can capture NTFF profiles via the
    injected libaxon_pjrt.so.  No-op if the real module exists or the .so
    is unavailable (grading path uses trace=False and never hits this)."""
    try:
        import antenv.axon_hooks  # noqa: F401

        return
    except Exception:
        pass
    import ctypes

    mod = types.ModuleType("antenv.axon_hooks")
    holder = [None, False]

    def set_axon_ntff_profile_hook(h):
        holder[0], holder[1] = h, True

    def _make_hook():
        try:
            lib = ctypes.CDLL("/opt/axon/libaxon_pjrt.so")
        except OSError:
            return None
        if not hasattr(lib, "axon_start_nrt_profile"):
            return None
        lib.axon_start_nrt_profile.argtypes = [
            ctypes.POINTER(ctypes.c_int64),
            ctypes.c_size_t,
        ]
        lib.axon_start_nrt_profile.restype = ctypes.c_int64
        lib.axon_stop_nrt_profile.argtypes = [ctypes.c_char_p]
        lib.axon_stop_nrt_profile.restype = ctypes.c_int64

        @contextlib.contextmanager
        def _hook(output_dir, device_ids):
            import jax

            jax.devices()
            if device_ids:
                ids = (ctypes.c_int64 * len(device_ids))(*device_ids)
                rc = lib.axon_start_nrt_profile(ids, len(device_ids))
            else:
                rc = lib.axon_start_nrt_profile(None, 0)
            if rc != 0:
                raise RuntimeError(f"axon_start_nrt_profile rc={rc}")
            try:
                yield
            finally:
                n = lib.axon_stop_nrt_profile(str(output_dir).encode())
                print(
                    f"profile: {n} file(s) written to {output_dir}",
                    file=sys.stderr,
                )

        return _hook

    def get_axon_ntff_profile_hook():
        if not holder[1]:
            holder[0], holder[1] = _make_hook(), True
        return holder[0]

    mod.set_axon_ntff_profile_hook = set_axon_ntff_profile_hook
    mod.get_axon_ntff_profile_hook = get_axon_ntff_profile_hook
    sys.modules["antenv.axon_hooks"] = mod


try:
    _install_ntff_shim()
except Exception:
    pass

import concourse.bass as bass
import concourse.mybir as mybir
import concourse.tile as tile
from concourse import masks
from concourse.bass_utils import run_bass_kernel_spmd
from bass_rust import ScopedClock

# ---------------------------------------------------------------- constants
C, I, O, D, K, B = 8, 2048, 16, 16, 8, 256
NCORES = 8
IL = I // NCORES          # 256 i's per core
F32 = mybir.dt.float32
F32R = mybir.dt.float32r
BF16 = mybir.dt.bfloat16
CB = C * B
KB_ = K * B               # 2048

# ------------------------------------------------- tile tail-drain workaround
_MAX_WAITS = 1


def _patched_drain_and_barrier(self, tick_clock, wait_clock):
    nc = self.nc
    drain_inst = nc.sync.drain()
    wait_clock.add_sem_waits(
        drain_inst.ins, ScopedClock({None: tick_clock.global_clock})
    )
    si = drain_inst.ins.sync_info
    if si is not None and si.on_wait and len(si.on_wait) > _MAX_WAITS:
        waits = list(si.on_wait)
        si.on_wait = waits[:_MAX_WAITS]
        for i in range(_MAX_WAITS, len(waits), _MAX_WAITS):
            extra = nc.sync.drain()
            extra.ins.sync_info = mybir.SyncInfo(
                on_wait=waits[i : i + _MAX_WAITS], on_update=[]
            )
    nc.all_engine_barrier()
    assert self.sems is not None
    popped = nc._tile_sem_poison_stack.pop()
    assert popped is self._sem_poison
    nc.clear_and_free_semaphores(list(self.sems.allocated().values()))
    nc.all_engine_barrier()


tile.TileContext._drain_and_barrier = _patched_drain_and_barrier

_fix_ctr = [0]


def fixup_multi_waits(nc):
    """walrus in this toolchain accepts at most one sem wait per instruction;
    hoist extra waits onto same-engine drains placed just before."""
    for f in nc.m.functions:
        for bb in f.blocks:
            out = []
            for inst in bb.instructions:
                si = inst.sync_info
                if si is not None and si.on_wait and len(si.on_wait) > _MAX_WAITS:
                    waits = list(si.on_wait)
                    for i in range(0, len(waits) - _MAX_WAITS, _MAX_WAITS):
                        _fix_ctr[0] += 1
                        d = mybir.InstDrain(
                            name=f"waitsplit_{_fix_ctr[0]}", ins=[], outs=[]
                        )
                        d.engine = inst.engine
                        d.sync_info = mybir.SyncInfo(
                            on_wait=waits[i : i + _MAX_WAITS], on_update=[]
                        )
                        out.append(d)
                    si.on_wait = waits[len(waits) - _MAX_WAITS :]
                out.append(inst)
            bb.instructions[:] = out
    return nc


def build_all(fixup=True):
    nc = bass.Bass("TRN2", target_bir_lowering=False, debug=False,
                   num_devices=NCORES)
    W_d = nc.dram_tensor("W", [C, IL, O, D, K], F32, kind="ExternalInput").ap()
    x_d = nc.dram_tensor("x", [B, IL, K], F32, kind="ExternalInput").ap()
    v_d = nc.dram_tensor("v", [C, O, B], F32R, kind="ExternalOutput").ap()
    xt_d = nc.dram_tensor("xt", [IL * K, B], BF16).ap()
    HB = CB // 2  # 4 classes per collective half
    cc_in = [[nc.dram_tensor(f"cc_in{t}_{u}", [16, HB], F32).ap()
              for u in range(2)] for t in range(3)]
    cc_out = [[nc.dram_tensor(f"cc_out{t}_{u}", [16, HB], F32).ap()
               for u in range(2)] for t in range(3)]

    with tile.TileContext(nc) as tc:
        with (
            tc.tile_pool(name="const", bufs=1) as constp,
            tc.tile_pool(name="persist", bufs=1) as pers,
            tc.tile_pool(name="small", bufs=4) as smallp,
        ):
            # ---------------- constants
            ident = constp.tile([128, 128], F32)
            masks.make_identity(nc, ident[:])
            identb = constp.tile([128, 128], BF16)
            with nc.allow_low_precision(reason="identity copy"):
                nc.vector.tensor_copy(identb[:], ident[:])
            ones16f = constp.tile([16, 1], F32)
            nc.gpsimd.memset(ones16f[:], 1.0)
            ones16 = constp.tile([16, 1], F32R)
            ones1f = constp.tile([1, 16], F32)
            nc.gpsimd.memset(ones1f[:], 1.0)
            ones1 = constp.tile([1, 16], F32R)
            with nc.allow_low_precision(reason="ones copy"):
                nc.vector.tensor_copy(ones16[:], ones16f[:])
                nc.vector.tensor_copy(ones1[:], ones1f[:])

            # ---------------- persistent state
            xt_i = pers.tile([128, 2 * KB_], BF16)      # [i, (h, k, b)]
            wf = pers.tile([128, 2 * C * 128], BF16)    # [i, (c, h, (k,o))]
            T2 = pers.tile([16, 2 * C * K * 128], BF16)  # [o, (c, h, k, i)]
            w_acc = pers.tile([16, CB], BF16)
            bt = pers.tile([128, 2 * CB], BF16)         # [i, (h, c, b)]
            e_all = pers.tile([128, 2 * CB], BF16)      # exp, then ct in-place

            # ------- allreduce + squash helper, one class-half at a time so
            # the collective for classes 0-3 overlaps compute of classes 4-7
            def allreduce_squash(t, u, pre, last, sqp):
                nm = f"{t}_{u}"
                nc.gpsimd.collective_compute(
                    "AllReduce",
                    mybir.AluOpType.add,
                    replica_groups=[list(range(NCORES))],
                    ins=[cc_in[t][u].opt()],
                    outs=[cc_out[t][u].opt()],
                )
                s_sum = sqp.tile([16, HB], F32, tag="s_sum", name=f"ss{nm}")
                nc.sync.dma_start(s_sum[:], cc_out[t][u][:, :])
                sq = sqp.tile([16, HB], F32R, tag="sq", name=f"sq{nm}")
                nc.scalar.activation(
                    sq[:], s_sum[:], mybir.ActivationFunctionType.Square,
                    scale=pre,
                )
                with tc.tile_pool(name=f"sqps{nm}", bufs=1,
                                  space="PSUM") as sqps:
                    ssq_ps = sqps.tile([1, HB], F32, tag="ssq")
                    for j in range(2):
                        nc.tensor.matmul(
                            ssq_ps[:, j * 512 : (j + 1) * 512],
                            ones16[:],
                            sq[:, j * 512 : (j + 1) * 512],
                            start=True, stop=True,
                        )
                    ssq_row = sqp.tile([1, HB], F32R, tag="row_tmp",
                                       name=f"ssq_row{nm}")
                    nc.scalar.copy(ssq_row[:], ssq_ps[:])
                ssq = sqp.tile([128, 8], F32R, tag="ssq_rs", name=f"ssqr{nm}")
                nc.sync.dma_start(
                    ssq[:], ssq_row[:].rearrange("u (p f) -> u p f", p=128)
                )
                den1 = sqp.tile([128, 8], F32, tag="den1", name=f"den1{nm}")
                nc.vector.tensor_scalar_add(den1[:], ssq[:], 1.0)
                r1 = sqp.tile([128, 8], F32, tag="r1", name=f"r1{nm}")
                nc.vector.reciprocal(r1[:], den1[:])
                rt = sqp.tile([128, 8], F32, tag="rt", name=f"rt{nm}")
                nc.scalar.sqrt(rt[:], ssq[:])
                r2 = sqp.tile([128, 8], F32, tag="r2", name=f"r2{nm}")
                nc.vector.reciprocal(r2[:], rt[:])
                m1 = sqp.tile([128, 8], F32, tag="m1", name=f"m1{nm}")
                nc.vector.tensor_mul(m1[:], ssq[:], r1[:])
                scale_rs = sqp.tile([128, 8], F32R, tag="scale_rs",
                                    name=f"srs{nm}")
                nc.vector.tensor_mul(scale_rs[:], m1[:], r2[:])
                if pre != 1.0:
                    nc.vector.tensor_scalar_mul(scale_rs[:], scale_rs[:], pre)
                scale_row = sqp.tile([1, HB], F32R, tag="row_tmp",
                                     name=f"srow{nm}")
                nc.sync.dma_start(
                    scale_row[:].rearrange("u (p f) -> u p f", p=128),
                    scale_rs[:],
                )
                with tc.tile_pool(name=f"bcps{nm}", bufs=1,
                                  space="PSUM") as bcps:
                    bc_ps = bcps.tile([16, HB], F32, tag="bc")
                    for j in range(2):
                        nc.tensor.matmul(
                            bc_ps[:, j * 512 : (j + 1) * 512],
                            ones1[:],
                            scale_row[:, j * 512 : (j + 1) * 512],
                            start=True, stop=True,
                        )
                    v_sb = sqp.tile([16, HB], F32R, tag="v_sbr",
                                    name=f"vsb{nm}")
                    with nc.allow_low_precision(reason="f32r full range"):
                        nc.vector.tensor_mul(v_sb[:], s_sum[:], bc_ps[:])
                    if last:
                        for cc in range(4):
                            nc.sync.dma_start(
                                v_d[u * 4 + cc],
                                v_sb[:, cc * B : (cc + 1) * B],
                            )
                    elif t == 0:
                        with nc.allow_low_precision(reason="w bf16"):
                            nc.vector.tensor_copy(
                                w_acc[:, u * HB : (u + 1) * HB], v_sb[:]
                            )
                    else:
                        with nc.allow_low_precision(reason="w accum"):
                            nc.vector.tensor_add(
                                w_acc[:, u * HB : (u + 1) * HB],
                                w_acc[:, u * HB : (u + 1) * HB],
                                v_sb[:],
                            )

            # ---------------- phase A: x -> xt_d -> xt_i
            phio_cm = contextlib.ExitStack()
            phio = phio_cm.enter_context(tc.tile_pool(name="phio", bufs=3))
            wtree = phio_cm.enter_context(tc.tile_pool(name="wtree", bufs=2))
            with tc.tile_pool(name="xps", bufs=4, space="PSUM") as xps:
                for bh in range(2):
                    xin = phio.tile([128, IL * K], F32, tag="xin", bufs=2)
                    nc.sync.dma_start(
                        xin[:],
                        x_d[bh * 128 : (bh + 1) * 128].rearrange(
                            "b i k -> b (i k)"
                        ),
                    )
                    xc = phio.tile([128, IL * K // 128 * 128], BF16, tag="xc",
                                   bufs=1)
                    for q in range(16):
                        ps = xps.tile([128, 128], F32)
                        nc.tensor.transpose(
                            ps[:], xin[:, q * 128 : (q + 1) * 128], ident[:]
                        )
                        nc.scalar.copy(xc[:, q * 128 : (q + 1) * 128], ps[:])
                    # dst rows (q*128+p), cols [bh*128, bh*128+128)
                    nc.scalar.dma_start(
                        xt_d.rearrange("(q p) b -> p q b", p=128)[
                            :, :, bh * 128 : (bh + 1) * 128
                        ],
                        xc[:].rearrange("p (q b) -> p q b", q=16),
                    )
            for h in range(2):
                # src rows i*8+k with i = h*128+p -> per partition 4KB run
                nc.sync.dma_start(
                    xt_i[:, h * KB_ : (h + 1) * KB_],
                    xt_d[h * 1024 : (h + 1) * 1024].rearrange(
                        "(p k) b -> p (k b)", k=K
                    ),
                )

            # ---------------- phase B: W -> wf (d-reduce trees) -> T, s0
            with (
                tc.tile_pool(name="tps", bufs=2, space="PSUM") as tpsp,
                tc.tile_pool(name="s0ps", bufs=2, space="PSUM") as s0ps,
                tc.tile_pool(name="sq0", bufs=1) as sqp0,
            ):
                for c in range(C):
                    for h in range(2):
                        t = 2 * c + h
                        wt = phio.tile([128, O * D * K], F32, tag="wt", bufs=2)
                        (nc.sync if h == 0 else nc.scalar).dma_start(
                            wt[:],
                            W_d[c, h * 128 : (h + 1) * 128].rearrange(
                                "p o d k -> p (o d k)"
                            ),
                        )
                        # reduce over d in 4 levels of strided adds.  All on
                        # DVE: gpsimd must stay empty before the first
                        # collective trigger or the cross-core barrier (and
                        # with it AllReduce 0) queues behind setup work.
                        eng = nc.vector
                        v4 = wt[:].rearrange("p (o d k) -> p o d k", o=O, d=D,
                                             k=K)
                        a1 = wtree.tile([128, 1024], F32, tag="a1")
                        a1v = a1[:].rearrange("p (o d k) -> p o d k", o=O, d=8,
                                              k=K)
                        eng.tensor_add(a1v, v4[:, :, 0:8, :], v4[:, :, 8:16, :])
                        a2 = wtree.tile([128, 512], F32, tag="a2")
                        a2v = a2[:].rearrange("p (o d k) -> p o d k", o=O, d=4,
                                              k=K)
                        eng.tensor_add(a2v, a1v[:, :, 0:4, :], a1v[:, :, 4:8, :])
                        a3 = wtree.tile([128, 256], F32, tag="a3")
                        a3v = a3[:].rearrange("p (o d k) -> p o d k", o=O, d=2,
                                              k=K)
                        eng.tensor_add(a3v, a2v[:, :, 0:2, :], a2v[:, :, 2:4, :])
                        # final: f32 -> bf16, output layout (k, o): o str 1, k str 16
                        wfs = wf[:, t * 128 : (t + 1) * 128].rearrange(
                            "p (k u o) -> p o u k", k=K, u=1
                        )
                        with nc.allow_low_precision(reason="wsum bf16"):
                            eng.tensor_add(
                                wfs, a3v[:, :, 0:1, :], a3v[:, :, 1:2, :]
                            )
                        # transpose each k-slice [128,16] -> [16,128] (base 0)
                        tp = tpsp.tile([16, K * 128], BF16, tag="tp")
                        for k in range(K):
                            nc.tensor.transpose(
                                tp[:, k * 128 : (k + 1) * 128],
                                wf[:, t * 128 + k * 16 : t * 128 + (k + 1) * 16],
                                identb[:],
                            )
                        nc.scalar.copy(
                            T2[:, t * K * 128 : (t + 1) * K * 128], tp[:]
                        )
                    # s0: uniform-c iteration 0 partials
                    s0p = s0ps.tile([16, B], F32, tag="s0p")
                    for h in range(2):
                        t = 2 * c + h
                        for k in range(K):
                            nc.tensor.matmul(
                                s0p[:],
                                wf[:, t * 128 + k * 16 : t * 128 + (k + 1) * 16],
                                xt_i[:, h * KB_ + k * B : h * KB_ + (k + 1) * B],
                                start=(h == 0 and k == 0),
                                stop=(h == 1 and k == K - 1),
                            )
                    s0sb = smallp.tile([16, B], F32, tag="s_sb",
                                       name=f"s0sb{c}")
                    nc.scalar.copy(s0sb[:], s0p[:])
                    nc.scalar.dma_start(
                        cc_in[0][c // 4][:, (c % 4) * B : (c % 4 + 1) * B],
                        s0sb[:],
                    )
                    if c == 3:
                        allreduce_squash(0, 0, 1.0 / C, last=False, sqp=sqp0)
                    elif c == 7:
                        allreduce_squash(0, 1, 1.0 / C, last=False, sqp=sqp0)

            phio_cm.close()

            # ---------------- routing iterations 1 and 2
            # DVE per-instruction overhead is ~0.4us, so everything below
            # works on the largest slices SBUF allows.
            with (
                tc.tile_pool(name="workp", bufs=1) as workp,
                tc.tile_pool(name="p2p", bufs=2) as p2p,
                tc.tile_pool(name="fldp", bufs=1) as fldp,
                tc.tile_pool(name="softp", bufs=1) as softp,
            ):
                for it in range(1, 3):
                    # ---- phase 1: bt[i, (h, c, b)] = sum_k xt*G
                    with tc.tile_pool(name=f"gps{it}", bufs=2,
                                      space="PSUM") as gps:
                        for h in range(2):
                            for cg in range(4):  # c-pairs
                                p2 = p2p.tile([128, 2 * KB_], BF16, tag="p2")
                                for cc in range(2):
                                    c = cg * 2 + cc
                                    t = 2 * c + h
                                    g = gps.tile([128, KB_], F32, tag="g")
                                    for k in range(K):
                                        nc.tensor.matmul(
                                            g[:, k * B : (k + 1) * B],
                                            T2[:, (t * K + k) * 128 :
                                               (t * K + k + 1) * 128],
                                            w_acc[:, c * B : (c + 1) * B],
                                            start=True, stop=True,
                                        )
                                    # Pool can't read PSUM: Act narrows G to
                                    # bf16 SBUF, then DVE/Pool go all-bf16.
                                    g16 = workp.tile([128, KB_], BF16,
                                                     tag="g16", bufs=3)
                                    nc.scalar.copy(g16[:], g[:])
                                    peng = (nc.gpsimd if t % 4 == 1
                                            else nc.vector)
                                    with nc.allow_low_precision(reason="P"):
                                        peng.tensor_mul(
                                            p2[:, cc * KB_ : (cc + 1) * KB_],
                                            xt_i[:, h * KB_ : (h + 1) * KB_],
                                            g16[:],
                                        )
                                # fold over k: 3 big strided adds per c-pair
                                p2v = p2[:].rearrange(
                                    "p (c k b) -> p c k b", c=2, k=K
                                )
                                fl1 = fldp.tile([128, KB_], BF16, tag="fl1")
                                f1v = fl1[:].rearrange(
                                    "p (c k b) -> p c k b", c=2, k=4
                                )
                                fl2 = fldp.tile([128, KB_ // 2], BF16,
                                                tag="fl2")
                                f2v = fl2[:].rearrange(
                                    "p (c k b) -> p c k b", c=2, k=2
                                )
                                btv = bt[:, h * CB + cg * 2 * B :
                                         h * CB + (cg * 2 + 2) * B] \
                                    .rearrange("p (c u b) -> p c u b",
                                               c=2, u=1)
                                with nc.allow_low_precision(reason="fold"):
                                    nc.vector.tensor_add(
                                        f1v, p2v[:, :, 0:4, :],
                                        p2v[:, :, 4:8, :]
                                    )
                                    nc.vector.tensor_add(
                                        f2v, f1v[:, :, 0:2, :],
                                        f1v[:, :, 2:4, :]
                                    )
                                    nc.vector.tensor_add(
                                        btv, f2v[:, :, 0:1, :],
                                        f2v[:, :, 1:2, :]
                                    )

                    # ---- phase 2: softmax over classes, both halves at
                    # once.  Iteration 1 logits are bounded by ~60 (|v0|<1),
                    # so exp() is safe without the max-subtraction there.
                    btv = bt[:].rearrange("p (h c b) -> p h c b", h=2, c=C)
                    m1 = softp.tile([128, 2 * 4 * B], BF16, tag="m1")
                    m1v = m1[:].rearrange("p (h c b) -> p h c b", h=2, c=4)
                    m2 = softp.tile([128, 2 * 2 * B], BF16, tag="m2")
                    m2v = m2[:].rearrange("p (h c b) -> p h c b", h=2, c=2)
                    rmax = softp.tile([128, 2 * B], BF16, tag="rmax")
                    rmv = rmax[:].rearrange("p (h u b) -> p h u b", h=2, u=1)
                    sub = softp.tile([128, 2 * CB], BF16, tag="sub")
                    subv = sub[:].rearrange("p (h c b) -> p h c b", h=2, c=C)
                    if True:
                        with nc.allow_low_precision(reason="softmax max"):
                            nc.vector.tensor_max(
                                m1v, btv[:, :, 0:4, :], btv[:, :, 4:8, :]
                            )
                            nc.vector.tensor_max(
                                m2v, m1v[:, :, 0:2, :], m1v[:, :, 2:4, :]
                            )
                            nc.vector.tensor_max(
                                rmv, m2v[:, :, 0:1, :], m2v[:, :, 1:2, :]
                            )
                            nc.vector.tensor_sub(
                                subv, btv,
                                rmax[:].rearrange("p (h b) -> p h b", h=2)
                                .unsqueeze(2).broadcast_to([128, 2, C, B]),
                            )
                        nc.scalar.activation(
                            e_all[:], sub[:],
                            mybir.ActivationFunctionType.Exp,
                        )
                    ev = e_all[:].rearrange("p (h c b) -> p h c b", h=2, c=C)
                    d1 = softp.tile([128, 2 * 4 * B], BF16, tag="m1",
                                    name=f"d1_{it}")
                    d1v = d1[:].rearrange("p (h c b) -> p h c b", h=2, c=4)
                    d2 = softp.tile([128, 2 * 2 * B], BF16, tag="m2",
                                    name=f"d2_{it}")
                    d2v = d2[:].rearrange("p (h c b) -> p h c b", h=2, c=2)
                    den = softp.tile([128, 2 * B], F32, tag="den")
                    denv = den[:].rearrange("p (h u b) -> p h u b", h=2, u=1)
                    with nc.allow_low_precision(reason="den partials bf16"):
                        nc.vector.tensor_add(
                            d1v, ev[:, :, 0:4, :], ev[:, :, 4:8, :]
                        )
                        nc.vector.tensor_add(
                            d2v, d1v[:, :, 0:2, :], d1v[:, :, 2:4, :]
                        )
                    nc.vector.tensor_add(
                        denv, d2v[:, :, 0:1, :], d2v[:, :, 1:2, :]
                    )
                    rec = softp.tile([128, 2 * B], F32, tag="rec")
                    nc.vector.reciprocal(rec[:], den[:])
                    recb = softp.tile([128, 2 * B], BF16, tag="recb")
                    nc.scalar.copy(recb[:], rec[:])
                    with nc.allow_low_precision(reason="rec bf16"):
                        # ct overwrites e in place
                        nc.vector.tensor_mul(
                            ev, ev,
                            recb[:].rearrange("p (h b) -> p h b", h=2)
                            .unsqueeze(2).broadcast_to([128, 2, C, B]),
                        )

                    # ---- phase 3: y = ct*x, s_c = sum_{h,k} wf^T y
                    with (
                        tc.tile_pool(name=f"sps{it}", bufs=1,
                                     space="PSUM") as sps,
                        tc.tile_pool(name=f"sq{it}", bufs=1) as sqpi,
                    ):
                        s_ps = sps.tile([16, CB], F32, tag="s")
                        for c in range(C):
                            y = workp.tile([128, 2 * KB_], BF16, tag="y",
                                           bufs=2)
                            yeng = nc.gpsimd if (c % 4) == 1 else nc.vector
                            with nc.allow_low_precision(reason="y bf16"):
                                yeng.tensor_mul(
                                    y[:].rearrange("p (h k b) -> p h k b",
                                                   h=2, k=K),
                                    xt_i[:].rearrange("p (h k b) -> p h k b",
                                                      h=2, k=K),
                                    e_all[:].rearrange(
                                        "p (h c b) -> p h c b", h=2, c=C
                                    )[:, :, c : c + 1, :]
                                    .broadcast_to([128, 2, K, B]),
                                )
                            for h in range(2):
                                t = 2 * c + h
                                for k in range(K):
                                    nc.tensor.matmul(
                                        s_ps[:, c * B : (c + 1) * B],
                                        wf[:, t * 128 + k * 16 :
                                           t * 128 + (k + 1) * 16],
                                        y[:, h * KB_ + k * B :
                                          h * KB_ + (k + 1) * B],
                                        start=(h == 0 and k == 0),
                                        stop=(h == 1 and k == K - 1),
                                    )
                            if c == 3 or c == 7:
                                u = c // 4
                                s_sb = smallp.tile(
                                    [16, HB], F32, tag="s_sbf",
                                    name=f"s_sb{it}_{u}", bufs=2,
                                )
                                nc.scalar.copy(
                                    s_sb[:], s_ps[:, u * HB : (u + 1) * HB]
                                )
                                nc.scalar.dma_start(cc_in[it][u][:, :],
                                                    s_sb[:])
                                allreduce_squash(
                                    it, u, 1.0, last=(it == 2), sqp=sqpi
                                )
    return fixup_multi_waits(nc) if fixup else nc


_NC = None


def kernel(x: np.ndarray, W: np.ndarray, _timings=None) -> np.ndarray:
    global _NC
    x = np.asarray(x, np.float32)
    W = np.asarray(W, np.float32)
    if _NC is None:
        _NC = build_all()
    in_maps = []
    for j in range(NCORES):
        sl = slice(j * IL, (j + 1) * IL)
        in_maps.append(
            {
                "W": np.ascontiguousarray(W[:, 0, sl]),
                "x": np.ascontiguousarray(x[:, sl, :]),
            }
        )
    res = run_bass_kernel_spmd(
        _NC, in_maps, core_ids=list(range(NCORES)),
        trace=_timings is not None,
    )
    if _timings is not None:
        _timings.append(res.exec_time_ns)
    v = res.results[0]["v"].astype(np.float32)  # [C, O, B]
    return np.ascontiguousarray(v.transpose(2, 0, 1))
